# revision 1
# baseline (speedup 1.0000x reference)
"""Trainium2 Bass kernel for nn_ContrastiveLoss (bs=128, nw=80, nf=64, d=768).

Strategy
--------
All four similarity paths of the module are slices of ONE augmented dot-product
tensor  G[t, wa, v, fa] = aug_w[t, wa] . aug_f[v, fa]  where
  aug_w = [word_features (80), sentence_output (1)]   (81 "words")
  aug_f = [frame_features (64), traj_output (1)]      (65 "frames")

  G[t, <80, v, <64] = S        (fine-grained word x frame)
  G[t, <80, v,  64] = A        (word x traj)
  G[t,  80, v, <64] = B        (sentence x frame)
  G[t,  80, v,  64] = traj_sent (exact)

With TAU = 0.01 every softmax-weighted pooling in the module is within
tau*ln(n) <= 0.05 of a plain max, and empirically the end-to-end loss differs
by ~1e-7 relative (measured against the f64 reference).  So:
  frame_word_sim[t,v]     ~ max_{w<80, f<64} G
  video_word_sim[t,v]     ~ max_{w<80} G[..., 64]
  sentence_frame_sim[t,v] ~ max_{f<64} G[t, 80, v, :]
This collapses the whole fine-grained path into max-reductions that are fused
directly onto the matmul's PSUM output - the [bs,nw,bs,nf] tensor never
touches HBM or even SBUF.

Sharding: videos are split 16-per-core across 8 cores (each core holds all
text). Each core produces a [128, 16] column block of the sim matrix, and the
full [128, 128] matrix is AllGathered for the exact (f32) bidirectional
cross-entropy, computed redundantly on every core.

Matmul layout (per core): stationary = aug_w k-chunk [128d, 128t] (one wa per
M-chunk, 81 chunks), moving = aug_f [128d, 520] per v-half (fa-major, so one
512-wide bank covers all fa<64 and an 8-wide block is the traj column).
bf16 operands, f32 PSUM accumulation over 6 k-chunks; the fa-max fuses onto
PSUM output per chunk and the wa-max accumulates across chunks in [128, v]
registers - the [bs,nw,bs,nf] tensor never exists in any memory.

Latency shaping: the sweep runs one v-half at a time with one AllGather per
half - AG#1 overlaps the half-1 sweep and re-syncs the ranks so AG#2 runs at
the mesh floor; half-0's CE statistics (row max/sumexp, per-column logsumexp,
diagonal partial) are computed under AG#2's shadow, leaving only half-1 stats
and a tiny logsumexp merge + partition-sum matmul after the last collective.
The wf operand streams in behind the compute (head/tail split DMA), and the
ACT Exp table is kept warm across the tail so only the Ln load remains.
"""

import os
import sys
from contextlib import ExitStack

import numpy as np
import ml_dtypes

_REPO = "/opt/trn_rl_repo"
if os.path.isdir(_REPO) and _REPO not in sys.path:
    sys.path.insert(0, _REPO)

BS, NW, NF, D, KC = 128, 80, 64, 768, 6
N_CORES = 8
V = BS // N_CORES            # videos per core = 16
WA = NW + 1                  # 81 augmented words (sentence last)
FAV = NF + 1                 # 65 augmented frames (traj last)
TAU = 0.01

_CACHE = {}


def _build_nc(v=V, n_cores=N_CORES):
    """Build + compile the SPMD per-core program (identical on all cores)."""
    from concourse import bacc, mybir, tile
    from concourse.tile import add_dep_helper

    F32 = mybir.dt.float32
    BF16 = mybir.dt.bfloat16
    AX = mybir.AxisListType.X
    ALU = mybir.AluOpType

    # moving side: two v-halves, each laid out fa-major (free idx = fa*hv + vl)
    # so one 512-wide bank covers all fa<64 and an hv-wide block is fa=64.
    hv = v // 2                          # videos per half = 8
    half_w = FAV * hv                    # 520
    free = 2 * half_w                    # 1040
    assert NF * hv == 512

    nc = bacc.Bacc(
        "TRN2", target_bir_lowering=False, debug=False, num_devices=n_cores
    )
    wfa_d = nc.dram_tensor("wfa", [KC, 128, WA * BS], BF16, kind="ExternalInput")
    ffa_d = nc.dram_tensor("ffa", [KC, 128, free], BF16, kind="ExternalInput")
    msk_d = nc.dram_tensor("msk", [128, 128], F32, kind="ExternalInput")
    loss_d = nc.dram_tensor("loss", [1, 1], F32, kind="ExternalOutput")
    sim_d = nc.dram_tensor("sim", [BS, v], F32, kind="ExternalOutput")

    with tile.TileContext(nc) as tc, ExitStack() as ctx:
        cpool = ctx.enter_context(tc.tile_pool(name="const", bufs=1))
        ps_pool = ctx.enter_context(tc.tile_pool(name="ps", bufs=5, space="PSUM"))
        psb_pool = ctx.enter_context(tc.tile_pool(name="psb", bufs=2, space="PSUM"))
        ps2_pool = ctx.enter_context(tc.tile_pool(name="ps2", bufs=1, space="PSUM"))
        tmp_pool = ctx.enter_context(tc.tile_pool(name="tmp", bufs=3))
        dram = ctx.enter_context(tc.tile_pool(name="dram", bufs=1, space="DRAM"))

        # DMA order: small ffa operands first, then a head slice of every wf
        # k-chunk (the first HEAD_M m-chunks' worth), then the remainders.
        # The PE can then start the sweep ~14us in and overlap the bulk load.
        HEAD1_M, HEAD_M = 2, 26
        h1 = HEAD1_M * BS
        head = HEAD_M * BS
        wf_sb, ff_sb = [], []
        for k in range(KC):
            t2 = cpool.tile([128, free], BF16, name=f"ff{k}")
            if k == 0:  # the very first matmul needs only these 0.2 MB
                nc.sync.dma_start(t2[:, :512], ffa_d.ap()[k][:, :512])
            else:
                nc.sync.dma_start(t2[:], ffa_d.ap()[k])
            ff_sb.append(t2)
            t = cpool.tile([128, WA * BS], BF16, name=f"wf{k}")
            nc.sync.dma_start(t[:, :h1], wfa_d.ap()[k][:, :h1])
            wf_sb.append(t)
        nc.sync.dma_start(ff_sb[0][:, 512:], ffa_d.ap()[0][:, 512:])
        for k in range(KC):
            nc.sync.dma_start(wf_sb[k][:, h1:head], wfa_d.ap()[k][:, h1:head])
        for k in range(KC):
            nc.sync.dma_start(wf_sb[k][:, head:], wfa_d.ap()[k][:, head:])
        msk_sb = cpool.tile([128, 128], F32, name="msk_sb")
        nc.gpsimd.dma_start(msk_sb[:], msk_d.ap())

        fw_acc = cpool.tile([128, v], F32, name="fw_acc")
        vw_acc = cpool.tile([128, v], F32, name="vw_acc")
        sf_acc = cpool.tile([128, v], F32, name="sf_acc")
        ts_acc = cpool.tile([128, v], F32, name="ts_acc")
        sim = cpool.tile([128, v], F32, name="simb")
        nc.vector.memset(fw_acc[:], -3.0e38)
        nc.vector.memset(vw_acc[:], -3.0e38)

        ag_in = [dram.tile([BS, hv], F32, name=f"ag_in{h}") for h in range(2)]
        ag_out = [
            dram.tile([n_cores, BS, hv], F32, name=f"ag_out{h}", addr_space="Shared")
            for h in range(2)
        ]

        # ---- main fused matmul + max sweep, one v-half per pass -----------
        # The half-0 AllGather overlaps the half-1 sweep (~half the kernel)
        # and doubles as a rank barrier, so the half-1 AllGather pays almost
        # no arrival skew.
        for h in range(2):
            base = h * half_w
            hs = slice(h * hv, (h + 1) * hv)
            for m in range(WA):
                psA = ps_pool.tile([128, 512], F32, tag="psA")
                for k in range(KC):
                    nc.tensor.matmul(
                        psA[:],
                        lhsT=wf_sb[k][:, m * BS : (m + 1) * BS],
                        rhs=ff_sb[k][:, base : base + 512],
                        start=(k == 0),
                        stop=(k == KC - 1),
                    )
                psB = psb_pool.tile([128, hv], F32, tag="psB")
                for k in range(KC):
                    nc.tensor.matmul(
                        psB[:],
                        lhsT=wf_sb[k][:, m * BS : (m + 1) * BS],
                        rhs=ff_sb[k][:, base + 512 : base + half_w],
                        start=(k == 0),
                        stop=(k == KC - 1),
                    )
                psA_v = psA[:].rearrange("p (fa vv) -> p vv fa", vv=hv)
                if m < NW:
                    t16 = tmp_pool.tile([128, hv], F32, tag="t16")
                    nc.vector.reduce_max(t16[:], psA_v, axis=AX)
                    nc.vector.tensor_max(fw_acc[:, hs], fw_acc[:, hs], t16[:])
                    nc.vector.tensor_max(vw_acc[:, hs], vw_acc[:, hs], psB[:])
                else:  # m == 80: sentence row
                    nc.vector.reduce_max(sf_acc[:, hs], psA_v, axis=AX)
                    nc.vector.tensor_copy(ts_acc[:, hs], psB[:])

            # combine this half's sim-block columns, kick its AllGather
            nc.vector.tensor_add(sim[:, hs], fw_acc[:, hs], sf_acc[:, hs])
            nc.vector.tensor_add(sim[:, hs], sim[:, hs], vw_acc[:, hs])
            nc.vector.tensor_add(sim[:, hs], sim[:, hs], ts_acc[:, hs])
            combine_inst = nc.vector.tensor_scalar_mul(sim[:, hs], sim[:, hs], 0.25)
            nc.sync.dma_start(ag_in[h][:], sim[:, hs])
            nc.gpsimd.collective_compute(
                "AllGather",
                ALU.bypass,
                replica_groups=[list(range(n_cores))],
                ins=[ag_in[h][:].opt()],
                outs=[ag_out[h][:].opt()],
            )

        nc.gpsimd.dma_start(sim_d.ap(), sim[:])

        # ---- exact bidirectional cross-entropy, split by column-half ------
        # Half h's gathered [128, 64] block holds full columns {16r + h*8+vl},
        # so its per-column (CE_col) stats and its row-partial (max/sumexp)
        # stats are final per half. Half 0's stats compute DURING the half-1
        # sweep (ACT/DVE are free); after AG#2 only half-1 stats + tiny
        # merges remain:
        #   loss = [ sum_t(Mrow + ln(e0*exp(mx0-Mrow) + e1*exp(mx1-Mrow)))
        #          + sum_h sum_v(mxc_h + ln(ec_h)) - 2*sum_t diag ] / 256
        HC = hv * n_cores                      # columns per half = 64
        ones = cpool.tile([128, 1], F32, name="ones")
        nc.gpsimd.memset(ones[:], 1.0)

        mxr = cpool.tile([128, 2], F32, name="mxr")    # row maxes per half
        nmxr = cpool.tile([128, 2], F32, name="nmxr")
        er = cpool.tile([128, 2], F32, name="er")      # row sumexp per half
        dgh = cpool.tile([128, 2], F32, name="dgh")    # diag parts per half
        mxc = cpool.tile([64, 2], F32, name="mxc")     # col maxes per half
        nmxc = cpool.tile([64, 2], F32, name="nmxc")
        ec = cpool.tile([64, 2], F32, name="ec")       # col sumexp per half
        lec = cpool.tile([64, 2], F32, name="lec")
        sLT = [None, None]

        for h in range(2):
            hh = slice(h, h + 1)
            sL = cpool.tile([128, HC], F32, name=f"simL{h}")
            g = nc.sync.dma_start(
                sL[:].rearrange("p (r vv) -> p r vv", r=n_cores),
                ag_out[h][:].rearrange("r p vv -> p r vv"),
            )
            if h == 0:
                # Order the half-0 CE chain after the sweep's last combine so
                # its DVE/ACT ops never head-of-line-block the sweep stream;
                # it then runs entirely under the half-1 AllGather's shadow.
                add_dep_helper(
                    g.ins, combine_inst.ins,
                    reason="defer CE-0 past the sweep",
                )
            nc.vector.reduce_max(mxr[:, hh], sL[:], axis=AX)
            nc.vector.tensor_scalar_mul(nmxr[:, hh], mxr[:, hh], -1.0)
            scr = tmp_pool.tile([128, HC], F32, tag="scr")
            nc.scalar.activation(
                scr[:], sL[:], mybir.ActivationFunctionType.Exp,
                bias=nmxr[:, hh], scale=1.0, accum_out=er[:, hh],
            )
            scr2 = tmp_pool.tile([128, HC], F32, tag="scr")
            nc.vector.tensor_mul(scr2[:], sL[:], msk_sb[:, h * HC : (h + 1) * HC])
            nc.vector.reduce_sum(dgh[:, hh], scr2[:], axis=AX)
            # full transpose = 32x32 DVE block transposes with swapped slices
            sLT[h] = cpool.tile([64, 128], F32, name=f"sLT{h}")
            for bi in range(4):
                for bj in range(2):
                    nc.vector.transpose(
                        sLT[h][32 * bj : 32 * bj + 32, 32 * bi : 32 * bi + 32],
                        sL[32 * bi : 32 * bi + 32, 32 * bj : 32 * bj + 32],
                    )
            nc.vector.reduce_max(mxc[:, hh], sLT[h][:], axis=AX)
            nc.vector.tensor_scalar_mul(nmxc[:, hh], mxc[:, hh], -1.0)
            scr3 = tmp_pool.tile([64, 128], F32, tag="scrT")
            nc.scalar.activation(
                scr3[:], sLT[h][:], mybir.ActivationFunctionType.Exp,
                bias=nmxc[:, hh], scale=1.0, accum_out=ec[:, hh],
            )

        # merge row stats across halves: e = sum_h er_h * exp(mxr_h - Mrow)
        Mrow = cpool.tile([128, 1], F32, name="Mrow")
        nMrow = cpool.tile([128, 1], F32, name="nMrow")
        dsc = cpool.tile([128, 2], F32, name="dsc")
        ew = cpool.tile([128, 2], F32, name="ew")
        es = cpool.tile([128, 1], F32, name="es")
        lser = cpool.tile([128, 1], F32, name="lser")
        nc.vector.tensor_max(Mrow[:], mxr[:, 0:1], mxr[:, 1:2])
        nc.vector.tensor_scalar_mul(nMrow[:], Mrow[:], -1.0)
        nc.scalar.activation(dsc[:], mxr[:], mybir.ActivationFunctionType.Exp,
                             bias=nMrow[:], scale=1.0)
        nc.vector.tensor_mul(ew[:], er[:], dsc[:])
        nc.vector.reduce_sum(es[:], ew[:], axis=AX)
        nc.scalar.activation(lec[:], ec[:], mybir.ActivationFunctionType.Ln)
        nc.scalar.activation(lser[:], es[:], mybir.ActivationFunctionType.Ln)

        # row vector: Mrow + lser - 2*(dg0 + dg1); col vector: mxc + lec summed
        dsum = cpool.tile([128, 1], F32, name="dsum")
        rv = cpool.tile([128, 1], F32, name="rv")
        nc.vector.reduce_sum(dsum[:], dgh[:], axis=AX)
        nc.vector.scalar_tensor_tensor(
            out=rv[:], in0=dsum[:], scalar=-2.0, in1=Mrow[:],
            op0=ALU.mult, op1=ALU.add,
        )
        nc.vector.tensor_add(rv[:], rv[:], lser[:])
        cv = cpool.tile([64, 1], F32, name="cv")
        cvb = cpool.tile([64, 1], F32, name="cvb")
        nc.vector.reduce_sum(cv[:], mxc[:], axis=AX)
        nc.vector.reduce_sum(cvb[:], lec[:], axis=AX)
        nc.vector.tensor_add(cv[:], cv[:], cvb[:])

        ps1 = ps2_pool.tile([1, 1], F32, tag="ps1")
        nc.tensor.matmul(ps1[:], lhsT=rv[:], rhs=ones[:], start=True, stop=False)
        nc.tensor.matmul(ps1[:], lhsT=cv[:], rhs=ones[0:64, :], start=False,
                         stop=True)
        lossv = cpool.tile([1, 1], F32, name="lossv")
        nc.vector.tensor_scalar_mul(lossv[:], ps1[:], 1.0 / (2.0 * BS))
        nc.sync.dma_start(loss_d.ap(), lossv[:])

    nc.compile()
    return nc


def _prep_in_maps(wf, ff, so, to, v=V, n_cores=N_CORES):
    """Host-side: build per-core bf16 operand arrays in matmul layout."""
    bf = ml_dtypes.bfloat16
    # stationary side: aug_w[t, wa, d] -> [d, wa, t] -> [KC, 128, WA*BS]
    aug_w = np.concatenate([wf, so[:, None, :]], axis=1)          # [BS, WA, D]
    wfa = np.ascontiguousarray(aug_w.transpose(2, 1, 0)).reshape(KC, 128, WA * BS)
    wfa = wfa.astype(bf)
    # moving side per core: two v-halves, each aug_f[vh, fa, d] -> [d, fa, vh]
    aug_f = np.concatenate([ff, to[:, None, :]], axis=1)          # [BS, FAV, D]
    hv = v // 2
    # block-diagonal masks: msk[16r + h*hv + vl, h*64 + r*hv + vl] = 1
    msk = np.zeros((128, 128), np.float32)
    for h in range(2):
        for r in range(n_cores):
            for vl in range(hv):
                msk[16 * r + h * hv + vl, h * 64 + r * hv + vl] = 1.0
    in_maps = []
    for c in range(n_cores):
        halves = []
        for h in range(2):
            blk = aug_f[c * v + h * hv : c * v + (h + 1) * hv]    # [hv, FAV, D]
            halves.append(
                np.ascontiguousarray(blk.transpose(2, 1, 0)).reshape(D, FAV * hv)
            )
        ffa = np.concatenate(halves, axis=1).reshape(KC, 128, FAV * v)
        in_maps.append({"wfa": wfa, "ffa": ffa.astype(bf), "msk": msk})
    return in_maps


def _run(in_maps, trace=False):
    from concourse.bass_utils import run_bass_kernel_spmd

    if "nc" not in _CACHE:
        _CACHE["nc"] = _build_nc()
    return run_bass_kernel_spmd(
        _CACHE["nc"], in_maps, core_ids=list(range(N_CORES)), trace=trace
    )


def _numpy_reference(traj_output, frame_features, sentence_output, word_features,
                     global_mat_weight, word_logit_weight, frame_logit_weight,
                     local_mat_weight, frame_mat_weight, word_mat_weight,
                     frame_mat_weight2, word_mat_weight2):
    """Exact f64 fallback (only used if the weight matrices are not identity)."""
    def softmax(x, axis):
        m = np.max(x, axis=axis, keepdims=True)
        e = np.exp(x - m)
        return e / np.sum(e, axis=axis, keepdims=True)

    def log_softmax(x, axis):
        m = np.max(x, axis=axis, keepdims=True)
        return x - m - np.log(np.sum(np.exp(x - m), axis=axis, keepdims=True))

    to = traj_output.astype(np.float64)
    ff = frame_features.astype(np.float64)
    so = sentence_output.astype(np.float64)
    wf = word_features.astype(np.float64)
    G, WL, FL = (global_mat_weight.astype(np.float64),
                 word_logit_weight.astype(np.float64),
                 frame_logit_weight.astype(np.float64))
    LM, FM, WM = (local_mat_weight.astype(np.float64),
                  frame_mat_weight.astype(np.float64),
                  word_mat_weight.astype(np.float64))
    FM2, WM2 = (frame_mat_weight2.astype(np.float64),
                word_mat_weight2.astype(np.float64))

    traj_sent = (so @ G) @ to.T
    A = np.einsum("twd,vd->twv", wf, to)
    sA = softmax(A / TAU, axis=1)
    wA = np.einsum("twv,wu->tuv", sA, WL)
    video_word = np.sum(A * wA, axis=1)
    B = np.einsum("td,vfd->vtf", so, ff)
    sB = softmax(B / TAU, axis=-1)
    sentence_frame = np.sum(B * (sB @ FL), axis=-1).T
    wfl = wf @ LM
    fw = np.zeros((BS, BS))
    for t in range(BS):
        S = np.einsum("wd,vfd->wvf", wfl[t], ff)
        sw = softmax(S / TAU, axis=0)
        word_level = np.sum(np.einsum("wvf,wu->uvf", sw, WM) * S, axis=0)
        sfx = softmax(S / TAU, axis=-1)
        frame_level = np.sum((sfx @ FM) * S, axis=-1)
        smw = softmax(word_level / TAU, axis=-1)
        s2f = np.sum((smw @ FM2) * word_level, axis=-1)
        smf = softmax(frame_level / TAU, axis=0)
        v2w = np.sum(np.einsum("wv,wu->uv", smf, WM2) * frame_level, axis=0)
        fw[t] = (s2f + v2w) / 2.0
    sim = (traj_sent + video_word + sentence_frame + fw) / 4.0

    def ce(m):
        return -np.mean(np.diagonal(log_softmax(m, -1)))

    return np.array((ce(sim) + ce(sim.T)) / 2.0, dtype=np.float32)


def kernel(**inputs):
    wf = np.ascontiguousarray(np.asarray(inputs["word_features"], np.float32))
    ff = np.ascontiguousarray(np.asarray(inputs["frame_features"], np.float32))
    so = np.ascontiguousarray(np.asarray(inputs["sentence_output"], np.float32))
    to = np.ascontiguousarray(np.asarray(inputs["traj_output"], np.float32))

    eye_names = [
        ("global_mat_weight", D), ("word_logit_weight", NW),
        ("frame_logit_weight", NF), ("local_mat_weight", D),
        ("frame_mat_weight", NF), ("word_mat_weight", NW),
        ("frame_mat_weight2", NF), ("word_mat_weight2", NW),
    ]
    for name, n in eye_names:
        w = np.asarray(inputs[name], np.float32)
        if not np.allclose(w, np.eye(n, dtype=np.float32), atol=1e-6):
            return _numpy_reference(**{k: np.asarray(x) for k, x in inputs.items()})

    res = _run(_prep_in_maps(wf, ff, so, to))
    return np.array(res.results[0]["loss"][0, 0], dtype=np.float32)



# revision 15
# speedup vs baseline: 1.7245x; 1.7245x over previous
"""Trainium2 Bass kernel for nn_ContrastiveLoss (bs=128, nw=80, nf=64, d=768).

Strategy (v2: word-sharded fp8 DoubleRow sweep)
-----------------------------------------------
All four similarity paths of the module are slices of ONE augmented dot-product
tensor  G[t, wa, v, fa] = aug_w[t, wa] . aug_f[v, fa]  where
  aug_w = [word_features (80), sentence_output (1)]
  aug_f = [frame_features (64), traj_output (1)]
With TAU = 0.01 every softmax-weighted pooling is within tau*ln(n) <= 0.05 of
a plain max (validated: end-to-end rel err ~1e-4), so the whole fine-grained
path collapses to max-reductions fused onto the matmul's PSUM output.

v2 sharding: the 80 WORDS are split 10-per-core across 8 cores; every core
holds ALL 128 videos. This gives each stationary [d,t] block 8320 moving
columns of reuse (vs 1040 under video-sharding), eliminating ~1000 tiny
8-column matmuls, and makes the cross-core combine an elementwise MAX:
  fw[t,v] = max_cores max_{m in core, fa} S     (AllReduce-max)
  vw[t,v] = max_cores max_{m in core} A         (same AllReduce)
The sentence row (sf = max_fa B, ts = sent.traj) is computed per-core for its
own 16 videos in bf16 and AllGathered (tiny, fully hidden).

The fine sweep runs in fp8 (float8e4) with MatmulPerfMode.DoubleRow: two
128-deep k-chunks per instruction at 2x FLOP rate. fp8 operand noise on a
768-deep dot is ~1.4 absolute on values whose pairwise gaps at the max are
~6+; measured end-to-end loss error is ~1e-4 (tolerance 2e-2).

Per half (64 videos) each core does 10 words x 8 blocks x 3 k-pair matmuls of
[128,512]; the fa-max reduce is split between the Vector engine (direct f32
PSUM reduce) and the Scalar engine (PSUM->SBUF bf16 convert, then a 4x-rate
bf16 DVE reduce) to balance engine load. The half-0 AllReduce(max) overlaps
the half-1 sweep; after the half-1 AllReduce only ~4us of cross-entropy
remains (both ACT tables are pre-warmed during the DMA ramp).
"""

import os
import sys
from contextlib import ExitStack

import numpy as np
import ml_dtypes

_REPO = "/opt/trn_rl_repo"
if os.path.isdir(_REPO) and _REPO not in sys.path:
    sys.path.insert(0, _REPO)

BS, NW, NF, D = 128, 80, 64, 768
N_CORES = 8
MPC = NW // N_CORES          # words per core = 10
NJ = 3                       # k-pairs (768 = 3 * 256)
FINE = BS * NF               # fine moving cols per k-plane = 8192
FREE8 = FINE + BS            # + 128 traj cols = 8320
TAU = 0.01
NEG = -3.0e38

_CACHE = {}


def _build_nc(n_cores=N_CORES):
    """Build + compile the SPMD per-core program (identical on all cores)."""
    from concourse import bacc, mybir, tile

    F32 = mybir.dt.float32
    BF16 = mybir.dt.bfloat16
    FP8 = mybir.dt.float8e4
    AX = mybir.AxisListType.X
    ALU = mybir.AluOpType
    ACT = mybir.ActivationFunctionType
    DR = mybir.MatmulPerfMode.DoubleRow

    nc = bacc.Bacc(
        "TRN2", target_bir_lowering=False, debug=False, num_devices=n_cores
    )
    # DoubleRow operands are plane-major [p, i, cols] (i = which 128-deep
    # k-chunk of the 256 pair); walrus requires unit-stride columns.
    #   wf8 cols: m*128 + t ; ff8 cols: v*64+fa (fine, vl-major) | 8192+v (traj)
    wf8_d = nc.dram_tensor("wf8", [NJ, 128, 2, MPC * 128], FP8, kind="ExternalInput")
    ff8_d = nc.dram_tensor("ff8", [NJ, 128, 2, FREE8], FP8, kind="ExternalInput")
    sent_d = nc.dram_tensor("sent", [6, 128, 128], BF16, kind="ExternalInput")
    ffv_d = nc.dram_tensor("ffv", [6, 128, 1040], BF16, kind="ExternalInput")
    eye_d = nc.dram_tensor("eye", [128, 128], F32, kind="ExternalInput")
    loss_d = nc.dram_tensor("loss", [1, 1], F32, kind="ExternalOutput")
    sim_d = nc.dram_tensor("sim", [BS, BS], F32, kind="ExternalOutput")

    with tile.TileContext(nc) as tc, ExitStack() as ctx:
        cpool = ctx.enter_context(tc.tile_pool(name="const", bufs=1))
        psA_pool = ctx.enter_context(tc.tile_pool(name="psA", bufs=6, space="PSUM"))
        psb_pool = ctx.enter_context(tc.tile_pool(name="psb", bufs=1, space="PSUM"))
        ps1_pool = ctx.enter_context(tc.tile_pool(name="ps1", bufs=1, space="PSUM"))
        tmp_pool = ctx.enter_context(tc.tile_pool(name="tmp", bufs=3))
        tmpb_pool = ctx.enter_context(tc.tile_pool(name="tmpb", bufs=4))
        dram = ctx.enter_context(tc.tile_pool(name="dram", bufs=1, space="DRAM"))

        # ---- DMA: half-0 j0 operands first so the PE starts ~4us in ------
        ff_sb = [cpool.tile([128, 2, FREE8], FP8, name=f"ff{j}") for j in range(NJ)]
        wf_sb = [cpool.tile([128, 2, MPC * 128], FP8, name=f"wf{j}") for j in range(NJ)]

        def ff_slice(j, c0, c1):  # cols [c0,c1) of both i-planes
            return ff_sb[j][:, :, c0:c1]

        def ff_dram_slice(j, c0, c1):
            return ff8_d.ap()[j][:, :, c0:c1]

        # half-0 fine (cols 0..4096) + half-0 traj (cols 8192..8256), j=0
        nc.sync.dma_start(ff_slice(0, 0, 4096), ff_dram_slice(0, 0, 4096))
        nc.sync.dma_start(ff_slice(0, FINE, FINE + 64), ff_dram_slice(0, FINE, FINE + 64))
        for j in range(NJ):
            nc.sync.dma_start(wf_sb[j][:], wf8_d.ap()[j])
        for j in range(1, NJ):
            nc.sync.dma_start(ff_slice(j, 0, 4096), ff_dram_slice(j, 0, 4096))
            nc.sync.dma_start(ff_slice(j, FINE, FINE + 64), ff_dram_slice(j, FINE, FINE + 64))
        sent_sb = [cpool.tile([128, 128], BF16, name=f"se{k}") for k in range(6)]
        ffv_sb = [cpool.tile([128, 1040], BF16, name=f"fv{k}") for k in range(6)]
        for k in range(6):
            nc.sync.dma_start(sent_sb[k][:], sent_d.ap()[k])
            nc.sync.dma_start(ffv_sb[k][:], ffv_d.ap()[k])
        eye_sb = cpool.tile([128, 128], F32, name="eye_sb")
        nc.gpsimd.dma_start(eye_sb[:], eye_d.ap())
        for j in range(NJ):  # half-1 fine + traj
            nc.sync.dma_start(ff_slice(j, 4096, FINE), ff_dram_slice(j, 4096, FINE))
            nc.sync.dma_start(
                ff_slice(j, FINE + 64, FREE8), ff_dram_slice(j, FINE + 64, FREE8)
            )

        # ---- accumulators + collective buffers ---------------------------
        fwvw = [cpool.tile([128, 128], F32, name=f"fwvw{h}") for h in range(2)]
        nc.vector.memset(fwvw[0][:], NEG)
        nc.vector.memset(fwvw[1][:], NEG)
        s2own = cpool.tile([128, 16], F32, name="s2own")

        ar_in = [dram.tile([128, 128], F32, name=f"ar_in{h}") for h in range(2)]
        ar_out = [
            dram.tile([128, 128], F32, name=f"ar_out{h}", addr_space="Shared")
            for h in range(2)
        ]
        ag_in = dram.tile([128, 16], F32, name="ag_in")
        ag_out = dram.tile([n_cores, 128, 16], F32, name="ag_out", addr_space="Shared")

        # pre-warm both ACT tables (Exp, Ln) so no table load lands in the
        # post-collective tail
        warm = cpool.tile([1, 1], F32, name="warm")
        nc.gpsimd.memset(warm[:], 1.0)
        warm2 = cpool.tile([1, 1], F32, name="warm2")
        nc.scalar.activation(warm2[:], warm[:], ACT.Exp)
        nc.scalar.activation(warm2[:], warm[:], ACT.Ln)

        def wf_ap(j, m):  # stationary [128, 2, 128] for word m, k-pair j
            return wf_sb[j][:, :, m * 128 : (m + 1) * 128]

        DIRECT = (0, 3, 6)  # blocks reduced straight from PSUM on the DVE

        def fine_half(h):
            c0 = h * 4096
            for m in range(MPC):
                fa_red = tmp_pool.tile([128, 64], F32, tag="fa_red")
                for pair in range(4):
                    ps = [
                        psA_pool.tile([128, 512], F32, tag="psA", name=f"psA_{q}")
                        for q in range(2)
                    ]
                    for j in range(NJ):
                        for q in range(2):
                            b = 2 * pair + q
                            nc.tensor.matmul(
                                ps[q][:],
                                lhsT=wf_ap(j, m),
                                rhs=ff_slice(j, c0 + b * 512, c0 + (b + 1) * 512),
                                start=(j == 0),
                                stop=(j == NJ - 1),
                                perf_mode=DR,
                            )
                    for q in range(2):
                        b = 2 * pair + q
                        pv = ps[q][:].rearrange("p (vl fa) -> p vl fa", vl=8)
                        if b in DIRECT:
                            nc.vector.reduce_max(fa_red[:, 8 * b : 8 * b + 8], pv, axis=AX)
                        else:
                            sc = tmpb_pool.tile([128, 512], BF16, tag="scb")
                            nc.scalar.activation(sc[:], ps[q][:], ACT.Copy)
                            nc.vector.reduce_max(
                                fa_red[:, 8 * b : 8 * b + 8],
                                sc[:].rearrange("p (vl fa) -> p vl fa", vl=8),
                                axis=AX,
                            )
                # traj cols for this half: A[t, v] partial
                psB = psb_pool.tile([128, 64], F32, tag="psB")
                tc0 = FINE + 64 * h
                for j in range(NJ):
                    nc.tensor.matmul(
                        psB[:],
                        lhsT=wf_ap(j, m),
                        rhs=ff_slice(j, tc0, tc0 + 64),
                        start=(j == 0),
                        stop=(j == NJ - 1),
                        perf_mode=DR,
                    )
                nc.vector.tensor_max(fwvw[h][:, 0:64], fwvw[h][:, 0:64], fa_red[:])
                nc.vector.tensor_max(fwvw[h][:, 64:128], fwvw[h][:, 64:128], psB[:])

        # ---- half 0 sweep, AllReduce(max) #0 ------------------------------
        fine_half(0)
        nc.sync.dma_start(ar_in[0][:], fwvw[0][:])
        nc.gpsimd.collective_compute(
            "AllReduce", ALU.max, replica_groups=[list(range(n_cores))],
            ins=[ar_in[0][:].opt()], outs=[ar_out[0][:].opt()],
        )

        # ---- sentence path (bf16, own 16 videos), AllGather ---------------
        psS = [
            psA_pool.tile([128, 512], F32, tag="psA", name=f"psS{q}")
            for q in range(2)
        ]
        psBs = psb_pool.tile([128, 64], F32, tag="psB")
        for k in range(6):
            for q in range(2):
                nc.tensor.matmul(
                    psS[q][:], lhsT=sent_sb[k][:],
                    rhs=ffv_sb[k][:, q * 512 : (q + 1) * 512],
                    start=(k == 0), stop=(k == 5),
                )
            nc.tensor.matmul(
                psBs[:, 0:16], lhsT=sent_sb[k][:], rhs=ffv_sb[k][:, 1024:1040],
                start=(k == 0), stop=(k == 5),
            )
        for q in range(2):
            nc.vector.reduce_max(
                s2own[:, 8 * q : 8 * q + 8],
                psS[q][:].rearrange("p (vl fa) -> p vl fa", vl=8),
                axis=AX,
            )
        nc.vector.tensor_add(s2own[:], s2own[:], psBs[:, 0:16])
        nc.sync.dma_start(ag_in[:], s2own[:])
        nc.gpsimd.collective_compute(
            "AllGather", ALU.bypass, replica_groups=[list(range(n_cores))],
            ins=[ag_in[:].opt()], outs=[ag_out[:].opt()],
        )

        # ---- half 1 sweep, AllReduce(max) #1 ------------------------------
        fine_half(1)
        nc.sync.dma_start(ar_in[1][:], fwvw[1][:])
        nc.gpsimd.collective_compute(
            "AllReduce", ALU.max, replica_groups=[list(range(n_cores))],
            ins=[ar_in[1][:].opt()], outs=[ar_out[1][:].opt()],
        )

        # ---- cross-entropy ------------------------------------------------
        # sim[:, 64h:64h+64] = (fw + vw + s2)/4 ; CE-0 runs under AR#1's
        # shadow, CE-1 + the tiny merge after it.
        s2full = cpool.tile([128, 128], F32, name="s2full")
        nc.sync.dma_start(
            s2full[:].rearrange("p (r c) -> p r c", r=n_cores),
            ag_out[:].rearrange("r p c -> p r c"),
        )
        ones = cpool.tile([128, 1], F32, name="ones")
        nc.gpsimd.memset(ones[:], 1.0)

        mxr = cpool.tile([128, 2], F32, name="mxr")
        nmxr = cpool.tile([128, 2], F32, name="nmxr")
        er = cpool.tile([128, 2], F32, name="er")
        dgh = cpool.tile([128, 2], F32, name="dgh")
        mxc = cpool.tile([64, 2], F32, name="mxc")
        nmxc = cpool.tile([64, 2], F32, name="nmxc")
        ec = cpool.tile([64, 2], F32, name="ec")
        lec = cpool.tile([64, 2], F32, name="lec")
        sL = [cpool.tile([128, 64], F32, name=f"sL{h}") for h in range(2)]

        for h in range(2):
            hh = slice(h, h + 1)
            arf = cpool.tile([128, 128], F32, name=f"arf{h}")
            nc.sync.dma_start(arf[:], ar_out[h][:])
            nc.vector.tensor_add(sL[h][:], arf[:, 0:64], arf[:, 64:128])
            nc.vector.tensor_add(sL[h][:], sL[h][:], s2full[:, 64 * h : 64 * h + 64])
            nc.vector.tensor_scalar_mul(sL[h][:], sL[h][:], 0.25)
            nc.gpsimd.dma_start(sim_d.ap()[:, 64 * h : 64 * h + 64], sL[h][:])

            nc.vector.reduce_max(mxr[:, hh], sL[h][:], axis=AX)
            nc.vector.tensor_scalar_mul(nmxr[:, hh], mxr[:, hh], -1.0)
            scr = tmp_pool.tile([128, 64], F32, tag="scr")
            nc.scalar.activation(
                scr[:], sL[h][:], ACT.Exp,
                bias=nmxr[:, hh], scale=1.0, accum_out=er[:, hh],
            )
            scr2 = tmp_pool.tile([128, 64], F32, tag="scr")
            nc.vector.tensor_mul(scr2[:], sL[h][:], eye_sb[:, 64 * h : 64 * h + 64])
            nc.vector.reduce_sum(dgh[:, hh], scr2[:], axis=AX)
            # column stats via 32x32 DVE block transposes
            sLT = cpool.tile([64, 128], F32, name=f"sLT{h}")
            for bi in range(4):
                for bj in range(2):
                    nc.vector.transpose(
                        sLT[32 * bj : 32 * bj + 32, 32 * bi : 32 * bi + 32],
                        sL[h][32 * bi : 32 * bi + 32, 32 * bj : 32 * bj + 32],
                    )
            nc.vector.reduce_max(mxc[:, hh], sLT[:], axis=AX)
            nc.vector.tensor_scalar_mul(nmxc[:, hh], mxc[:, hh], -1.0)
            scr3 = tmp_pool.tile([64, 128], F32, tag="scrT")
            nc.scalar.activation(
                scr3[:], sLT[:], ACT.Exp,
                bias=nmxc[:, hh], scale=1.0, accum_out=ec[:, hh],
            )

        # merge row stats across halves; assemble loss
        Mrow = cpool.tile([128, 1], F32, name="Mrow")
        nMrow = cpool.tile([128, 1], F32, name="nMrow")
        dsc = cpool.tile([128, 2], F32, name="dsc")
        ew = cpool.tile([128, 2], F32, name="ew")
        es = cpool.tile([128, 1], F32, name="es")
        lser = cpool.tile([128, 1], F32, name="lser")
        nc.vector.tensor_max(Mrow[:], mxr[:, 0:1], mxr[:, 1:2])
        nc.vector.tensor_scalar_mul(nMrow[:], Mrow[:], -1.0)
        nc.scalar.activation(dsc[:], mxr[:], ACT.Exp, bias=nMrow[:], scale=1.0)
        nc.vector.tensor_mul(ew[:], er[:], dsc[:])
        nc.vector.reduce_sum(es[:], ew[:], axis=AX)
        nc.scalar.activation(lec[:], ec[:], ACT.Ln)
        nc.scalar.activation(lser[:], es[:], ACT.Ln)

        dsum = cpool.tile([128, 1], F32, name="dsum")
        rv = cpool.tile([128, 1], F32, name="rv")
        nc.vector.reduce_sum(dsum[:], dgh[:], axis=AX)
        nc.vector.scalar_tensor_tensor(
            out=rv[:], in0=dsum[:], scalar=-2.0, in1=Mrow[:],
            op0=ALU.mult, op1=ALU.add,
        )
        nc.vector.tensor_add(rv[:], rv[:], lser[:])
        cv = cpool.tile([64, 1], F32, name="cv")
        cvb = cpool.tile([64, 1], F32, name="cvb")
        nc.vector.reduce_sum(cv[:], mxc[:], axis=AX)
        nc.vector.reduce_sum(cvb[:], lec[:], axis=AX)
        nc.vector.tensor_add(cv[:], cv[:], cvb[:])

        ps1 = ps1_pool.tile([1, 1], F32, tag="ps1")
        nc.tensor.matmul(ps1[:], lhsT=rv[:], rhs=ones[:], start=True, stop=False)
        nc.tensor.matmul(ps1[:], lhsT=cv[:], rhs=ones[0:64, :], start=False, stop=True)
        lossv = cpool.tile([1, 1], F32, name="lossv")
        nc.vector.tensor_scalar_mul(lossv[:], ps1[:], 1.0 / (2.0 * BS))
        nc.sync.dma_start(loss_d.ap(), lossv[:])

    nc.compile()
    return nc


def _prep_in_maps(wf, ff, so, to, n_cores=N_CORES):
    """Host-side: build per-core operand arrays in matmul layout."""
    fp8 = ml_dtypes.float8_e4m3
    bf = ml_dtypes.bfloat16

    # stationary: wf8[j, p, i, (m, t)] = wf[t, wa=10c+m, d=256j+128i+p]
    W = np.ascontiguousarray(wf.transpose(2, 1, 0)).astype(fp8)   # [d, wa, t]
    W = W.reshape(NJ, 2, 128, NW, BS)                             # [j, i, p, wa, t]

    # moving: ff8[j, p, i, col]; col = v*64+fa | 8192+v
    F = np.ascontiguousarray(ff.transpose(2, 0, 1)).astype(fp8)   # [d, v, fa]
    F = F.reshape(NJ, 2, 128, FINE)                               # [j, i, p, v*64+fa]
    T8 = np.ascontiguousarray(to.T).astype(fp8).reshape(NJ, 2, 128, BS)
    FT = np.concatenate([F, T8], axis=3)                          # [j, i, p, 8320]
    ff8 = np.ascontiguousarray(FT.transpose(0, 2, 1, 3))          # [j, p, i, 8320]

    # sentence stationary: sent[k, p, t] = so[t, 128k+p]
    sent = np.ascontiguousarray(so.T).astype(bf).reshape(6, 128, 128)

    Fb = np.ascontiguousarray(ff.transpose(2, 0, 1)).astype(bf)   # [d, v, fa]
    Fb = Fb.reshape(6, 128, BS, NF)
    Tb = np.ascontiguousarray(to.T).astype(bf).reshape(6, 128, BS)

    eye = np.eye(128, dtype=np.float32)

    in_maps = []
    for c in range(n_cores):
        wc = W[:, :, :, MPC * c : MPC * (c + 1), :]               # [j,i,p,10,t]
        wf8 = np.ascontiguousarray(wc.transpose(0, 2, 1, 3, 4)).reshape(
            NJ, 128, 2, MPC * 128
        )
        ffv = np.concatenate(
            [
                Fb[:, :, 16 * c : 16 * c + 16, :].reshape(6, 128, 1024),
                Tb[:, :, 16 * c : 16 * c + 16],
            ],
            axis=2,
        )
        in_maps.append(
            {"wf8": wf8, "ff8": ff8, "sent": sent,
             "ffv": np.ascontiguousarray(ffv), "eye": eye}
        )
    return in_maps


def _run(in_maps, trace=False):
    from concourse.bass_utils import run_bass_kernel_spmd

    if "nc" not in _CACHE:
        _CACHE["nc"] = _build_nc()
    return run_bass_kernel_spmd(
        _CACHE["nc"], in_maps, core_ids=list(range(N_CORES)), trace=trace
    )


def _numpy_reference(traj_output, frame_features, sentence_output, word_features,
                     global_mat_weight, word_logit_weight, frame_logit_weight,
                     local_mat_weight, frame_mat_weight, word_mat_weight,
                     frame_mat_weight2, word_mat_weight2):
    """Exact f64 fallback (only used if the weight matrices are not identity)."""
    def softmax(x, axis):
        m = np.max(x, axis=axis, keepdims=True)
        e = np.exp(x - m)
        return e / np.sum(e, axis=axis, keepdims=True)

    def log_softmax(x, axis):
        m = np.max(x, axis=axis, keepdims=True)
        return x - m - np.log(np.sum(np.exp(x - m), axis=axis, keepdims=True))

    to = traj_output.astype(np.float64)
    ff = frame_features.astype(np.float64)
    so = sentence_output.astype(np.float64)
    wf = word_features.astype(np.float64)
    G, WL, FL = (global_mat_weight.astype(np.float64),
                 word_logit_weight.astype(np.float64),
                 frame_logit_weight.astype(np.float64))
    LM, FM, WM = (local_mat_weight.astype(np.float64),
                  frame_mat_weight.astype(np.float64),
                  word_mat_weight.astype(np.float64))
    FM2, WM2 = (frame_mat_weight2.astype(np.float64),
                word_mat_weight2.astype(np.float64))

    traj_sent = (so @ G) @ to.T
    A = np.einsum("twd,vd->twv", wf, to)
    sA = softmax(A / TAU, axis=1)
    wA = np.einsum("twv,wu->tuv", sA, WL)
    video_word = np.sum(A * wA, axis=1)
    B = np.einsum("td,vfd->vtf", so, ff)
    sB = softmax(B / TAU, axis=-1)
    sentence_frame = np.sum(B * (sB @ FL), axis=-1).T
    wfl = wf @ LM
    fw = np.zeros((BS, BS))
    for t in range(BS):
        S = np.einsum("wd,vfd->wvf", wfl[t], ff)
        sw = softmax(S / TAU, axis=0)
        word_level = np.sum(np.einsum("wvf,wu->uvf", sw, WM) * S, axis=0)
        sfx = softmax(S / TAU, axis=-1)
        frame_level = np.sum((sfx @ FM) * S, axis=-1)
        smw = softmax(word_level / TAU, axis=-1)
        s2f = np.sum((smw @ FM2) * word_level, axis=-1)
        smf = softmax(frame_level / TAU, axis=0)
        v2w = np.sum(np.einsum("wv,wu->uv", smf, WM2) * frame_level, axis=0)
        fw[t] = (s2f + v2w) / 2.0
    sim = (traj_sent + video_word + sentence_frame + fw) / 4.0

    def ce(m):
        return -np.mean(np.diagonal(log_softmax(m, -1)))

    return np.array((ce(sim) + ce(sim.T)) / 2.0, dtype=np.float32)


def kernel(**inputs):
    wf = np.ascontiguousarray(np.asarray(inputs["word_features"], np.float32))
    ff = np.ascontiguousarray(np.asarray(inputs["frame_features"], np.float32))
    so = np.ascontiguousarray(np.asarray(inputs["sentence_output"], np.float32))
    to = np.ascontiguousarray(np.asarray(inputs["traj_output"], np.float32))

    eye_names = [
        ("global_mat_weight", D), ("word_logit_weight", NW),
        ("frame_logit_weight", NF), ("local_mat_weight", D),
        ("frame_mat_weight", NF), ("word_mat_weight", NW),
        ("frame_mat_weight2", NF), ("word_mat_weight2", NW),
    ]
    for name, n in eye_names:
        w = np.asarray(inputs[name], np.float32)
        if not np.allclose(w, np.eye(n, dtype=np.float32), atol=1e-6):
            return _numpy_reference(**{k: np.asarray(x) for k, x in inputs.items()})

    res = _run(_prep_in_maps(wf, ff, so, to))
    return np.array(res.results[0]["loss"][0, 0], dtype=np.float32)
